# revision 28
# baseline (speedup 1.0000x reference)
"""GAT (3-layer, 8-head) forward on 8 Trainium2 NeuronCores — v2.

Strategy (graph/data parallel, per sharding hint):
  - Nodes sharded across 8 cores (2500 -> padded 2560 rows each).
  - All tensor data on device is fp16 (PSUM accumulation fp32): 4x faster
    PE matmuls vs fp32, 2x DVE, half the DMA/collective traffic.
  - Per layer, per 128-node block: h = x @ W and e = x @ (W A) via
    feature-major GEMMs; [h | e_src] rows (1152B, 256B-aligned) DMA'd to a
    DRAM shard and AllGather'd in NPIECE pieces (piece-major row ids) so the
    collective overlaps the previous layer's edge phase.
  - Edge phase per dst block: ONE batched dma_gather (int16 idxs) pulls all
    of the block's source rows (amortizes the ~1us SWDGE fixed cost, spreads
    across 4 SWDGE queues); 0/1 scatter masks are built on-chip from dst-id
    compares (iota vs per-chunk dst column) instead of DMA'd from the host;
    segment reduction of [ee*h | ee] happens on the TensorEngine into PSUM.
  - Softmax normalization post-aggregation (exact up to fp reassociation;
    |e| is O(1) so no max-subtraction needed).
  - LayerNorm epilogue avoids Sqrt (its ACT table set clashes with Exp):
    rstd = exp(-0.5*ln(var+eps)); all ACT functions used (Exp, Prelu, Ln,
    Identity, Relu, Copy) live in one table set -> no ACT_TABLE_LOAD thrash.
  - Next layer's GEMM + AllGather pieces are emitted interleaved with the
    edge loop so the collectives hide behind compute.
  - Mean-pool by graph via host-built 0/1 masks + AllReduce, then FC head.
"""

import os
import sys

sys.path.insert(0, "/opt/trn_rl_repo")

import numpy as np

import concourse.bass as bass
import concourse.mybir as mybir
import concourse.tile as tile
from concourse import bacc
from concourse.bass_utils import run_bass_kernel_spmd
from concourse.masks import make_identity

F32 = mybir.dt.float32
F16 = mybir.dt.float16
I16 = mybir.dt.int16
ALU = mybir.AluOpType
ACT = mybir.ActivationFunctionType

P = 128

# Results of the last run (for test harnesses).
LAST_RESULTS = None


def _full_cfg():
    return dict(
        n_cores=8,
        N=20000,
        D=512,
        H=8,
        G=128,
        OUT=128,
        NEG=0.2,
        EPS=1e-5,
        L=3,
        NPIECE=2,
        NQ=4,  # SWDGE queues for the gathers
        ROWF=640,  # padded gather row (fp16 elems); 640*2 = 1280B, 256B-aligned
    )


# --------------------------------------------------------------------------
# Host-side preprocessing: pure index manipulation / relayout (dtype casts
# and zero padding only).
# --------------------------------------------------------------------------


def _prep(inputs, cfg):
    nc_ = cfg["n_cores"]
    N, D, H, G, OUT, L = cfg["N"], cfg["D"], cfg["H"], cfg["G"], cfg["OUT"], cfg["L"]
    NPIECE, ROWF = cfg["NPIECE"], cfg["ROWF"]
    C = D // H

    x = np.asarray(inputs["x"], np.float32)
    ei = np.asarray(inputs["edge_index"])
    batch = np.asarray(inputs["batch"]).astype(np.int64)

    SHR = (N + nc_ - 1) // nc_  # real nodes per core
    SH = ((SHR + P - 1) // P) * P  # padded nodes per core
    NB = SH // P
    KD = D // P
    PR = SH // NPIECE  # rows per AllGather piece

    # --- edges with self loops
    loops = np.arange(N, dtype=np.int64)
    src = np.concatenate([ei[0], loops])
    dst = np.concatenate([ei[1], loops])

    # piece-major global row id of a node (matches the piecewise AllGather)
    def pid_of(n):
        c = n // SHR
        r = n % SHR
        p = r // PR
        return p * (nc_ * PR) + c * PR + (r % PR)

    spid = pid_of(src)
    spiece = (src % SHR) // PR  # which AllGather piece holds the source row
    srel = spid - spiece * (nc_ * PR)  # row id within the piece's table
    dcore = dst // SHR
    dloc_all = dst % SHR

    # --- group edges by (core, dst block, src piece); dst-sorted per group
    per_core_blocks = [
        [[None] * NPIECE for _ in range(NB)] for _ in range(nc_)
    ]  # [core][block][piece] -> (src_rel, dst_local_in_block)
    nmax = np.zeros((NB, NPIECE), np.int64)
    for c in range(nc_):
        sel = np.nonzero(dcore == c)[0]
        dl = dloc_all[sel]
        blk = dl // P
        for b in range(NB):
            m = blk == b
            sb_, db_, pb_ = srel[sel[m]], dl[m] % P, spiece[sel[m]]
            for p in range(NPIECE):
                mp = pb_ == p
                s_b, d_b = sb_[mp], db_[mp]
                o = np.argsort(d_b, kind="stable")
                per_core_blocks[c][b][p] = (s_b[o], d_b[o])
                nmax[b, p] = max(nmax[b, p], mp.sum())

    # chunks per (block, piece), shared across cores; each piece gathered
    # separately so its chunk range starts on a chunk boundary
    CHP = np.maximum(1, -(-nmax // P)).astype(int)  # [NB, NPIECE]
    CH = CHP.sum(1)  # total chunks per block
    OFF = np.concatenate([[0], np.cumsum(CH)]).astype(int)
    TOTC = int(OFF[-1])
    CHMAX = int(CH.max())

    gidx = np.zeros((nc_, P, 8 * TOTC), np.int16)
    dlocs = np.full((nc_, P, TOTC), 1000.0, np.float16)
    # per-dst [start, end) slot ranges per (block, piece) (dst-sorted edges;
    # slot ids are global within the block)
    st = np.zeros((nc_, P, NPIECE * NB), np.float32)
    en = np.zeros((nc_, P, NPIECE * NB), np.float32)
    for c in range(nc_):
        for b in range(NB):
            c0 = 0
            for p in range(NPIECE):
                s_p, d_l = per_core_blocks[c][b][p]
                n = len(s_p)
                nid = CHP[b, p] * P
                idxs = np.zeros(nid, np.int64)
                idxs[:n] = s_p
                j = np.arange(nid)
                # idx block wrapped into 16 partitions, replicated across
                # the 8 GpSimd Q7 cores
                coloff = 8 * (OFF[b] + c0)
                for k in range(8):
                    gidx[c, 16 * k + (j % 16), coloff + j // 16] = idxs
                jr = np.arange(n)
                dlocs[c, jr % P, OFF[b] + c0 + jr // P] = d_l
                cnt = np.bincount(d_l, minlength=P)
                ends = np.cumsum(cnt)
                base = c0 * P
                st[c, :, p * NB + b] = base + ends - cnt
                en[c, :, p * NB + b] = base + ends
                c0 += CHP[b, p]

    # --- x shards, feature-major (KD, 128, SH) fp16
    xT = np.zeros((nc_, KD, P, SH), np.float16)
    for c in range(nc_):
        rows = x[c * SHR : min((c + 1) * SHR, N)]
        xp = np.zeros((SH, D), np.float32)
        xp[: len(rows)] = rows
        xT[c] = xp.T.reshape(KD, P, SH).astype(np.float16)

    # --- weights (fp16)
    W_all = np.zeros((L, KD, P, D), np.float16)
    WT_all = np.zeros((L, KD, P, D), np.float16)
    A_all = np.zeros((L, KD, P, 2 * H), np.float16)
    b_l, g_l, be_l = [], [], []
    for l in range(L):
        W = np.asarray(inputs[f"W{l}"], np.float32)
        W_all[l] = W.reshape(KD, P, D).astype(np.float16)
        WT_all[l] = np.ascontiguousarray(W.T).reshape(KD, P, D).astype(np.float16)
        A = np.zeros((D, 2 * H), np.float32)
        a_s = np.asarray(inputs[f"as{l}"], np.float32)
        a_d = np.asarray(inputs[f"ad{l}"], np.float32)
        for h in range(H):
            A[h * C : (h + 1) * C, h] = a_s[h]
            A[h * C : (h + 1) * C, H + h] = a_d[h]
        A_all[l] = A.reshape(KD, P, 2 * H).astype(np.float16)
        b_l.append(np.asarray(inputs[f"b{l}"], np.float32))
        g_l.append(np.asarray(inputs[f"g{l}"], np.float32))
        be_l.append(np.asarray(inputs[f"be{l}"], np.float32))

    skip_b = all(not b.any() for b in b_l)
    skip_g = all((g == 1.0).all() for g in g_l)
    skip_be = all(not be.any() for be in be_l)

    fc_W = np.asarray(inputs["fc_W"], np.float32).reshape(KD, P, OUT)
    fc_Wr = np.ascontiguousarray(fc_W.transpose(1, 0, 2)).astype(np.float16)
    fc_b = np.asarray(inputs["fc_b"], np.float32)
    skip_fcb = not fc_b.any()

    # --- pool masks (0/1 membership) fp16; counts precomputed on host
    poolmask = np.zeros((nc_, NB, P, G), np.float16)
    for c in range(nc_):
        lo = c * SHR
        hi = min((c + 1) * SHR, N)
        loc = np.arange(hi - lo)
        g_of = batch[lo:hi]
        poolmask[c, loc // P, loc % P, g_of] = 1.0
    counts = np.bincount(batch, minlength=G).astype(np.float32)
    crec = (1.0 / np.maximum(counts, 1.0)).reshape(G, 1)

    meta = dict(
        SH=SH, NB=NB, KD=KD, PR=PR, CH=CH, CHP=CHP, OFF=OFF, TOTC=TOTC,
        CHMAX=CHMAX,
        skip_b=skip_b, skip_g=skip_g, skip_be=skip_be, skip_fcb=skip_fcb,
    )

    in_maps = []
    for c in range(nc_):
        m = dict(
            xT=xT[c],
            W_all=W_all,
            WT_all=WT_all,
            A_all=A_all,
            fc_W=fc_Wr,
            gidx=gidx[c],
            dloc=dlocs[c],
            st=st[c],
            en=en[c],
            poolmask=poolmask[c],
            crec=crec,
        )
        if not skip_b:
            m["b_rep"] = np.broadcast_to(
                np.stack(b_l)[:, None, :], (L, P, D)
            ).copy()
        if not skip_g:
            m["g_rep"] = np.broadcast_to(
                np.stack(g_l)[:, None, :], (L, P, D)
            ).copy()
        if not skip_be:
            m["be_rep"] = np.broadcast_to(
                np.stack(be_l)[:, None, :], (L, P, D)
            ).copy()
        if not skip_fcb:
            m["fcb_rep"] = np.broadcast_to(fc_b[None, :], (P, OUT)).copy()
        in_maps.append(m)
    return in_maps, meta


# --------------------------------------------------------------------------
# Device program
# --------------------------------------------------------------------------


def build(tc, cfg, meta, I, out_ap):
    nc = tc.nc
    nc_cores = cfg["n_cores"]
    D, H, G, OUT, L = cfg["D"], cfg["H"], cfg["G"], cfg["OUT"], cfg["L"]
    NEG, EPS = cfg["NEG"], cfg["EPS"]
    NPIECE, NQ, ROWF = cfg["NPIECE"], cfg["NQ"], cfg["ROWF"]
    SH, NB, KD, PR = meta["SH"], meta["NB"], meta["KD"], meta["PR"]
    CH, OFF, TOTC, CHMAX = meta["CH"], meta["OFF"], meta["TOTC"], meta["CHMAX"]
    CHP = meta["CHP"]
    H2 = 2 * H
    BPP = NB // NPIECE  # blocks per AllGather piece

    rg = [list(range(nc_cores))]
    shared = "Shared" if nc_cores > 4 else "Local"

    from contextlib import ExitStack

    ctx = ExitStack()
    res = ctx.enter_context(tc.tile_pool(name="res", bufs=1))
    dram = ctx.enter_context(tc.tile_pool(name="dram", bufs=1, space="DRAM"))
    psum = ctx.enter_context(tc.tile_pool(name="psum", bufs=1, space="PSUM"))
    sb = ctx.enter_context(tc.tile_pool(name="sb", bufs=1))

    # ---------------- resident tiles
    xT_sb = [res.tile([P, SH], F16, name=f"xT{k}") for k in range(KD)]
    xn_sb = [res.tile([P, D], F16, name=f"xn{b}") for b in range(NB)]
    henm_sb = [res.tile([P, H2], F16, name=f"henm{b}") for b in range(NB)]
    W_sb = [res.tile([P, D], F16, name=f"W{k}") for k in range(KD)]
    WT_sb = [res.tile([P, D], F16, name=f"WT{k}") for k in range(KD)]
    A_sb = [res.tile([P, H2], F16, name=f"A{k}") for k in range(KD)]
    wa_sb = [res.tile([P, H2], F16, name=f"wa{k}") for k in range(KD)]
    idx_sb = res.tile([P, 8 * TOTC], I16, name="idx_sb")
    dloc_sb = res.tile([P, TOTC], F16, name="dloc_sb")
    st_sb = res.tile([P, NPIECE * NB], F32, name="st_sb")
    en_sb = res.tile([P, NPIECE * NB], F32, name="en_sb")
    id128 = res.tile([P, P], F16, name="id128")
    make_identity(nc, id128[:])
    # iota3[p, a, b] = b   (dst-id pattern for the mask equality compare)
    iota3 = res.tile([P, CHMAX, P], F16, name="iota3")
    nc.gpsimd.iota(
        iota3[:], pattern=[[0, CHMAX], [1, P]], base=0, channel_multiplier=0,
        allow_small_or_imprecise_dtypes=True,
    )
    # iotaG[p, a, b] = a*128 + b  (edge-slot id for the staircase compare)
    iotaG = res.tile([P, CHMAX, P], F32, name="iotaG")
    nc.gpsimd.iota(
        iotaG[:], pattern=[[P, CHMAX], [1, P]], base=0, channel_multiplier=0,
        allow_small_or_imprecise_dtypes=True,
    )

    b_rep = g_rep = be_rep = None
    if not meta["skip_b"]:
        b_rep = res.tile([P, D], F32, name="b_rep")
    if not meta["skip_g"]:
        g_rep = res.tile([P, D], F32, name="g_rep")
    if not meta["skip_be"]:
        be_rep = res.tile([P, D], F32, name="be_rep")

    nc.sync.dma_start(out=idx_sb[:], in_=I["gidx"][:])
    nc.sync.dma_start(out=dloc_sb[:], in_=I["dloc"][:])
    nc.sync.dma_start(out=st_sb[:], in_=I["st"][:])
    nc.sync.dma_start(out=en_sb[:], in_=I["en"][:])
    for k in range(KD):
        nc.sync.dma_start(out=xT_sb[k][:], in_=I["xT"][k])

    # ---------------- DRAM comm buffers
    ag_in = [
        [dram.tile([PR, ROWF], F16, name=f"ag_in{l}_{p}") for p in range(NPIECE)]
        for l in range(L)
    ]
    ag_out = [
        [
            dram.tile(
                [nc_cores * PR, ROWF], F16, name=f"ag_out{l}_{p}",
                addr_space=shared,
            )
            for p in range(NPIECE)
        ]
        for l in range(L)
    ]
    ar_in = dram.tile([G, D], F32, name="ar_in")
    ar_out = dram.tile([G, D], F32, name="ar_out", addr_space=shared)

    # ---------------- helpers
    def load_weights(l):
        for k in range(KD):
            nc.sync.dma_start(out=W_sb[k][:], in_=I["W_all"][l, k])
            nc.sync.dma_start(out=WT_sb[k][:], in_=I["WT_all"][l, k])
            nc.sync.dma_start(out=A_sb[k][:], in_=I["A_all"][l, k])
        if b_rep is not None:
            nc.sync.dma_start(out=b_rep[:], in_=I["b_rep"][l])
        if g_rep is not None:
            nc.sync.dma_start(out=g_rep[:], in_=I["g_rep"][l])
        if be_rep is not None:
            nc.sync.dma_start(out=be_rep[:], in_=I["be_rep"][l])

    def compute_wa():
        # wa = W @ A, feature-major chunks (KD x [128, 2H])
        for ic in range(KD):
            wa_ps = psum.tile([P, H2], F32, name="wa_ps", tag="ed", bufs=2)
            for oc in range(KD):
                nc.tensor.matmul(
                    out=wa_ps[:],
                    lhsT=WT_sb[oc][:, ic * P : (ic + 1) * P],
                    rhs=A_sb[oc][:],
                    start=(oc == 0),
                    stop=(oc == KD - 1),
                )
            nc.vector.tensor_copy(out=wa_sb[ic][:], in_=wa_ps[:])

    def gemm_block(l, b):
        # h (node-major) and e=[e_src|e_dst] for block b -> ag_in row piece
        bs = slice(b * P, (b + 1) * P)
        h_ps = psum.tile([P, D], F32, name="h_ps", tag="big", bufs=2)
        for k in range(KD):
            nc.tensor.matmul(
                out=h_ps[:], lhsT=xT_sb[k][:, bs], rhs=W_sb[k][:],
                start=(k == 0), stop=(k == KD - 1),
            )
        he_ps = psum.tile([P, H2], F32, name="he_ps", tag="ed", bufs=2)
        for k in range(KD):
            nc.tensor.matmul(
                out=he_ps[:], lhsT=xT_sb[k][:, bs], rhs=wa_sb[k][:],
                start=(k == 0), stop=(k == KD - 1),
            )
        nc.vector.tensor_copy(out=henm_sb[b][:], in_=he_ps[:])
        hrow = sb.tile([P, ROWF], F16, name="hrow", tag="hrow", bufs=3)
        nc.scalar.activation(hrow[:, 0:D], h_ps[:], ACT.Copy)
        nc.vector.tensor_copy(out=hrow[:, D : D + H], in_=he_ps[:, 0:H])
        nc.vector.memset(hrow[:, D + H : ROWF], 0.0)
        p = b // BPP
        rr = (b % BPP) * P
        nc.scalar.dma_start(out=ag_in[l][p][rr : rr + P, :], in_=hrow[:])

    def ag_piece(l, p):
        nc.gpsimd.collective_compute(
            "AllGather",
            ALU.bypass,
            replica_groups=rg,
            ins=[ag_in[l][p][:].opt()],
            outs=[ag_out[l][p][:].opt()],
        )

    def edge_block(l, b):
        chb = int(CH[b])
        off = int(OFF[b])
        gt = sb.tile([P, CHMAX, ROWF], F16, name="gt", tag="gt", bufs=3)
        # one gather per AllGather piece (each <= 1024 idxs, the SWDGE cap)
        c0 = 0
        for p in range(NPIECE):
            c1 = c0 + int(CHP[b, p])
            nidx = (c1 - c0) * P
            assert nidx <= 1024
            nc.gpsimd.dma_gather(
                out_ap=gt[:, c0:c1, :],
                in_ap=ag_out[l][p][:],
                idxs_ap=idx_sb[:, 8 * (off + c0) : 8 * (off + c1)],
                num_idxs=nidx,
                num_idxs_reg=nidx,
                elem_size=ROWF,
                queue_num=((NPIECE * b + p) % NQ),
            )
            c0 = c1
        gs = sb.tile([P, CHMAX, D + H], F16, name="gs", tag="gs", bufs=3)
        out_ps = psum.tile([P, D], F32, name="out_ps", tag="big", bufs=2)
        den_ps = psum.tile([P, H], F32, name="den_ps", tag="den", bufs=1)
        # masks for all chunks of the block, built on DVE:
        #   mk[jj, ch, d]  = (dloc[jj, ch] == d)
        #   mkT[d, ch, jj] = (st[d] <= ch*128+jj < en[d])   (edges dst-sorted)
        mk_all = sb.tile([P, CHMAX, P], F16, name="mk_all", tag="mk_all", bufs=3)
        nc.vector.tensor_tensor(
            out=mk_all[:, 0:chb, :],
            in0=iota3[:, 0:chb, :],
            in1=dloc_sb[:, off : off + chb].unsqueeze(2).to_broadcast(
                [P, chb, P]
            ),
            op=ALU.is_equal,
        )
        mkT_all = sb.tile([P, CHMAX, P], F16, name="mkT_all", tag="mkT_all", bufs=3)
        c0 = 0
        for p in range(NPIECE):
            c1 = c0 + int(CHP[b, p])
            col = p * NB + b
            nc.vector.tensor_scalar(
                out=mkT_all[:, c0:c1, :], in0=iotaG[:, c0:c1, :],
                scalar1=st_sb[:, col : col + 1], scalar2=None, op0=ALU.is_ge,
            )
            nc.vector.scalar_tensor_tensor(
                out=mkT_all[:, c0:c1, :], in0=iotaG[:, c0:c1, :],
                scalar=en_sb[:, col : col + 1], in1=mkT_all[:, c0:c1, :],
                op0=ALU.is_lt, op1=ALU.mult,
            )
            c0 = c1
        # e_dst per edge slot via mask matmuls into one PSUM strip
        ed_all = psum.tile([P, CHMAX * H], F32, name="ed_all", tag="ed", bufs=2)
        for ch in range(chb):
            nc.tensor.matmul(
                out=ed_all[:, ch * H : (ch + 1) * H],
                lhsT=mkT_all[:, ch, :], rhs=henm_sb[b][:, H:H2],
                start=True, stop=True, skip_group_check=True,
            )
        e_all = sb.tile([P, CHMAX, H], F16, name="e_all", tag="e_all", bufs=3)
        nc.vector.tensor_tensor(
            out=e_all[:, 0:chb, :], in0=gt[:, 0:chb, D : D + H],
            in1=ed_all[:, 0 : chb * H].rearrange("p (a h) -> p a h", h=H),
            op=ALU.add,
        )
        pr_all = sb.tile([P, CHMAX, H], F16, name="pr_all", tag="pr_all", bufs=3)
        nc.scalar.activation(
            pr_all[:, 0:chb, :], e_all[:, 0:chb, :], ACT.Prelu, alpha=NEG
        )
        nc.scalar.activation(gs[:, 0:chb, D : D + H], pr_all[:, 0:chb, :], ACT.Exp)
        mul_eng = nc.vector if (b % 2 == 0) else nc.gpsimd
        mul_eng.tensor_tensor(
            out=gs[:, 0:chb, 0:D].rearrange("p a (h c) -> p a h c", h=H),
            in0=gt[:, 0:chb, 0:D].rearrange("p a (h c) -> p a h c", h=H),
            in1=gs[:, 0:chb, D : D + H].unsqueeze(3).to_broadcast(
                [P, chb, H, D // H]
            ),
            op=ALU.mult,
        )
        for ch in range(chb):
            nc.tensor.matmul(
                out=den_ps[:], lhsT=mk_all[:, ch, :], rhs=gs[:, ch, D : D + H],
                start=(ch == 0), stop=(ch == chb - 1),
            )
        for ch in range(chb):
            nc.tensor.matmul(
                out=out_ps[:], lhsT=mk_all[:, ch, :], rhs=gs[:, ch, 0:D],
                start=(ch == 0), stop=(ch == chb - 1),
            )

        # ----- block epilogue: normalize by segment softmax denom, LN, relu
        den_sb = sb.tile([P, H], F32, name="den_sb", tag="den_sb", bufs=2)
        nc.vector.tensor_scalar_add(out=den_sb[:], in0=den_ps[:], scalar1=1e-16)
        rec = sb.tile([P, H], F32, name="rec", tag="rec", bufs=2)
        nc.vector.reciprocal(out=rec[:], in_=den_sb[:])
        y_sb = sb.tile([P, D], F32, name="y_sb", tag="y_sb", bufs=2)
        nc.vector.tensor_tensor(
            out=y_sb[:].rearrange("p (h c) -> p h c", h=H),
            in0=out_ps[:].rearrange("p (h c) -> p h c", h=H),
            in1=rec[:].unsqueeze(2).to_broadcast([P, H, D // H]),
            op=ALU.mult,
        )
        if b_rep is not None:
            nc.vector.tensor_add(out=y_sb[:], in0=y_sb[:], in1=b_rep[:])
        # mean and variance sums on the ACT engine (it has spare capacity)
        sq16 = sb.tile([P, D], F16, name="sq16", tag="sq16", bufs=2)
        sy = sb.tile([P, 1], F32, name="sy", tag="sy", bufs=2)
        nc.scalar.activation(sq16[:], y_sb[:], ACT.Copy, accum_out=sy[:, 0:1])
        nmu = sb.tile([P, 1], F32, name="nmu", tag="nmu", bufs=2)
        nc.scalar.mul(nmu[:], sy[:], -1.0 / D)  # nmu = -mu
        ssq = sb.tile([P, 1], F32, name="ssq", tag="ssq", bufs=2)
        nc.scalar.activation(
            sq16[:], y_sb[:], ACT.Square, bias=nmu[:, 0:1],
            accum_out=ssq[:, 0:1],
        )
        # rstd = 1/sqrt(ssq/D + eps) on DVE via bit-trick + 2 Newton steps
        # (avoids Sqrt/Ln on the ACT engine, whose tables clash with Exp)
        vv = sb.tile([P, 1], F32, name="vv", tag="vv", bufs=2)
        nc.vector.tensor_scalar(
            out=vv[:], in0=ssq[:], scalar1=1.0 / D, scalar2=float(EPS),
            op0=ALU.mult, op1=ALU.add,
        )
        ri = sb.tile([P, 1], mybir.dt.int32, name="ri", tag="ri", bufs=2)
        nc.vector.tensor_scalar(
            out=ri[:], in0=vv[:].bitcast(mybir.dt.int32), scalar1=1,
            scalar2=-1, op0=ALU.logical_shift_right, op1=ALU.bitwise_xor,
        )
        nc.vector.tensor_scalar_add(out=ri[:], in0=ri[:], scalar1=0x5F3759DF + 1)
        rstd = sb.tile([P, 1], F32, name="rstd", tag="rstd", bufs=2)
        nc.vector.tensor_copy(out=rstd[:], in_=ri[:].bitcast(F32))
        for _ in range(2):  # 2 Newton steps: rel err ~5e-6
            nr_a = sb.tile([P, 1], F32, name="nr_a", tag="nr_a", bufs=2)
            nc.vector.tensor_mul(out=nr_a[:], in0=rstd[:], in1=rstd[:])
            nc.vector.tensor_mul(out=nr_a[:], in0=nr_a[:], in1=vv[:])
            nc.vector.tensor_scalar(
                out=nr_a[:], in0=nr_a[:], scalar1=-0.5, scalar2=1.5,
                op0=ALU.mult, op1=ALU.add,
            )
            nc.vector.tensor_mul(out=rstd[:], in0=rstd[:], in1=nr_a[:])
        mm = sb.tile([P, 1], F32, name="mm", tag="mm", bufs=2)
        nc.vector.tensor_mul(out=mm[:], in0=nmu[:], in1=rstd[:])
        if g_rep is None and be_rep is None:
            nc.scalar.activation(
                xn_sb[b][:], y_sb[:], ACT.Relu,
                scale=rstd[:, 0:1], bias=mm[:, 0:1],
            )
        else:
            ln_sb = sb.tile([P, D], F32, name="ln_sb", tag="ln_sb", bufs=2)
            nc.scalar.activation(
                ln_sb[:], y_sb[:], ACT.Identity,
                scale=rstd[:, 0:1], bias=mm[:, 0:1],
            )
            if g_rep is not None:
                nc.vector.tensor_mul(out=ln_sb[:], in0=ln_sb[:], in1=g_rep[:])
            if be_rep is not None:
                nc.vector.tensor_add(out=ln_sb[:], in0=ln_sb[:], in1=be_rep[:])
            nc.scalar.activation(xn_sb[b][:], ln_sb[:], ACT.Relu)

    def trans_block(b):
        # xn block -> feature-major xT for the next layer's GEMM
        for k in range(KD):
            t_ps = psum.tile([P, P], F16, name="t_ps", tag="tr", bufs=2)
            nc.tensor.transpose(
                out=t_ps[:], in_=xn_sb[b][:, k * P : (k + 1) * P],
                identity=id128[:],
            )
            nc.vector.tensor_copy(
                out=xT_sb[k][:, b * P : (b + 1) * P], in_=t_ps[:]
            )

    # ---------------- program
    load_weights(0)
    compute_wa()
    for b in range(NB):
        gemm_block(0, b)
        if (b + 1) % BPP == 0:
            ag_piece(0, (b + 1) // BPP - 1)

    pm_pool = ctx.enter_context(tc.tile_pool(name="pm", bufs=2))
    pool_ps = psum.tile([G, D], F32, name="pool_ps", tag="pool", bufs=1)

    def pool_block(b):
        pm_sb = pm_pool.tile([P, G], F16, name="pm_sb", tag="pm_sb", bufs=2)
        nc.scalar.dma_start(out=pm_sb[:], in_=I["poolmask"][b])
        nc.tensor.matmul(
            out=pool_ps[:], lhsT=pm_sb[:], rhs=xn_sb[b][:],
            start=(b == 0), stop=(b == NB - 1),
        )

    for l in range(L):
        if l + 1 < L:
            load_weights(l + 1)
            compute_wa()
        for b in range(NB):
            edge_block(l, b)
            if l + 1 < L:
                trans_block(b)
                gemm_block(l + 1, b)
                if (b + 1) % BPP == 0:
                    ag_piece(l + 1, (b + 1) // BPP - 1)
            else:
                pool_block(b)

    # ---------------- pooling epilogue (counts precomputed on host) + FC
    pool_sb = res.tile([G, D], F32, name="pool_sb")
    nc.vector.tensor_copy(out=pool_sb[:], in_=pool_ps[:])
    nc.sync.dma_start(out=ar_in[:], in_=pool_sb[:])
    nc.gpsimd.collective_compute(
        "AllReduce",
        ALU.add,
        replica_groups=rg,
        ins=[ar_in[:].opt()],
        outs=[ar_out[:].opt()],
    )
    pf_sb = res.tile([G, D], F32, name="pf_sb")
    nc.sync.dma_start(out=pf_sb[:], in_=ar_out[:])
    crec_sb = res.tile([G, 1], F32, name="crec_sb")
    nc.sync.dma_start(out=crec_sb[:], in_=I["crec"][:])
    pn16 = res.tile([G, D], F16, name="pn16")
    nc.vector.tensor_tensor(
        out=pn16[:], in0=pf_sb[:],
        in1=crec_sb[:].to_broadcast([G, D]), op=ALU.mult,
    )
    # transpose pooled -> (KD chunks of (128, G))
    pT_sb = res.tile([P, KD, G], F16, name="pT_sb")
    for k in range(KD):
        t2_ps = psum.tile([P, G], F16, name="t2_ps", tag="tr", bufs=2)
        nc.tensor.transpose(
            out=t2_ps[:], in_=pn16[:, k * P : (k + 1) * P], identity=id128[:]
        )
        nc.vector.tensor_copy(out=pT_sb[:, k, :], in_=t2_ps[:])
    fcw_sb = res.tile([P, KD, OUT], F16, name="fcw_sb")
    nc.sync.dma_start(out=fcw_sb[:], in_=I["fc_W"][:])
    fc_ps = psum.tile([G, OUT], F32, name="fc_ps", tag="big", bufs=2)
    for k in range(KD):
        nc.tensor.matmul(
            out=fc_ps[:], lhsT=pT_sb[:, k, :], rhs=fcw_sb[:, k, :],
            start=(k == 0), stop=(k == KD - 1),
        )
    o_sb = res.tile([G, OUT], F32, name="o_sb")
    if not meta["skip_fcb"]:
        fcb_rep = res.tile([P, OUT], F32, name="fcb_rep")
        nc.sync.dma_start(out=fcb_rep[:], in_=I["fcb_rep"][:])
        nc.vector.tensor_add(out=o_sb[:], in0=fc_ps[:], in1=fcb_rep[0:G, :])
    else:
        nc.vector.tensor_copy(out=o_sb[:], in_=fc_ps[:])
    nc.sync.dma_start(out=out_ap[:], in_=o_sb[:])
    ctx.close()


# --------------------------------------------------------------------------
# Entry point
# --------------------------------------------------------------------------


def kernel(**inputs):
    global LAST_RESULTS
    cfg = _full_cfg()
    in_maps, meta = _prep(inputs, cfg)

    nc = bacc.Bacc(
        "TRN2",
        target_bir_lowering=False,
        debug=False,
        enable_asserts=False,
        num_devices=cfg["n_cores"],
        num_swdge_queues=cfg["NQ"],
    )
    I = {}
    for name, arr in in_maps[0].items():
        I[name] = nc.dram_tensor(
            name, arr.shape, mybir.dt.from_np(arr.dtype), kind="ExternalInput"
        ).ap()
    out_ap = nc.dram_tensor(
        "out", (cfg["G"], cfg["OUT"]), F32, kind="ExternalOutput"
    ).ap()

    with tile.TileContext(nc) as tc:
        build(tc, cfg, meta, I, out_ap)
    nc.compile()

    trace = bool(int(os.environ.get("GAT_TRACE", "0")))
    res = run_bass_kernel_spmd(
        nc,
        in_maps,
        core_ids=list(range(cfg["n_cores"])),
        trace=trace,
    )
    LAST_RESULTS = res
    return np.asarray(res.results[0]["out"])


# revision 30
# speedup vs baseline: 1.2569x; 1.2569x over previous
"""GAT (3-layer, 8-head) forward on 8 Trainium2 NeuronCores — v2.

Strategy (graph/data parallel, per sharding hint):
  - Nodes sharded across 8 cores (2500 -> padded 2560 rows each).
  - All tensor data on device is fp16 (PSUM accumulation fp32): 4x faster
    PE matmuls vs fp32, 2x DVE, half the DMA/collective traffic.
  - Per layer, per 128-node block: h = x @ W and e = x @ (W A) via
    feature-major GEMMs; [h | e_src] rows (1152B, 256B-aligned) DMA'd to a
    DRAM shard and AllGather'd in NPIECE pieces (piece-major row ids) so the
    collective overlaps the previous layer's edge phase.
  - Edge phase per dst block: ONE batched dma_gather (int16 idxs) pulls all
    of the block's source rows (amortizes the ~1us SWDGE fixed cost, spreads
    across 4 SWDGE queues); 0/1 scatter masks are built on-chip from dst-id
    compares (iota vs per-chunk dst column) instead of DMA'd from the host;
    segment reduction of [ee*h | ee] happens on the TensorEngine into PSUM.
  - Softmax normalization post-aggregation (exact up to fp reassociation;
    |e| is O(1) so no max-subtraction needed).
  - LayerNorm epilogue avoids Sqrt (its ACT table set clashes with Exp):
    rstd = exp(-0.5*ln(var+eps)); all ACT functions used (Exp, Prelu, Ln,
    Identity, Relu, Copy) live in one table set -> no ACT_TABLE_LOAD thrash.
  - Next layer's GEMM + AllGather pieces are emitted interleaved with the
    edge loop so the collectives hide behind compute.
  - Mean-pool by graph via host-built 0/1 masks + AllReduce, then FC head.
"""

import os
import sys

sys.path.insert(0, "/opt/trn_rl_repo")

import numpy as np

import concourse.bass as bass
import concourse.mybir as mybir
import concourse.tile as tile
from concourse import bacc
from concourse.bass_utils import run_bass_kernel_spmd
from concourse.masks import make_identity

F32 = mybir.dt.float32
F16 = mybir.dt.float16
I16 = mybir.dt.int16
ALU = mybir.AluOpType
ACT = mybir.ActivationFunctionType

P = 128

# Results of the last run (for test harnesses).
LAST_RESULTS = None


def _full_cfg():
    return dict(
        n_cores=8,
        N=20000,
        D=512,
        H=8,
        G=128,
        OUT=128,
        NEG=0.2,
        EPS=1e-5,
        L=3,
        NPIECE=2,
        NQ=4,  # SWDGE queues for the gathers
        ROWF=640,  # padded gather row (fp16 elems); 640*2 = 1280B, 256B-aligned
    )


# --------------------------------------------------------------------------
# Host-side preprocessing: pure index manipulation / relayout (dtype casts
# and zero padding only).
# --------------------------------------------------------------------------


def _prep(inputs, cfg):
    nc_ = cfg["n_cores"]
    N, D, H, G, OUT, L = cfg["N"], cfg["D"], cfg["H"], cfg["G"], cfg["OUT"], cfg["L"]
    NPIECE, ROWF = cfg["NPIECE"], cfg["ROWF"]
    C = D // H

    x = np.asarray(inputs["x"], np.float32)
    ei = np.asarray(inputs["edge_index"])
    batch = np.asarray(inputs["batch"]).astype(np.int64)

    SHR = (N + nc_ - 1) // nc_  # real nodes per core
    SH = ((SHR + P - 1) // P) * P  # padded nodes per core
    NB = SH // P
    KD = D // P
    PR = SH // NPIECE  # rows per AllGather piece

    # --- edges with self loops
    loops = np.arange(N, dtype=np.int64)
    src = np.concatenate([ei[0], loops])
    dst = np.concatenate([ei[1], loops])

    # piece-major global row id of a node (matches the piecewise AllGather)
    def pid_of(n):
        c = n // SHR
        r = n % SHR
        p = r // PR
        return p * (nc_ * PR) + c * PR + (r % PR)

    spid = pid_of(src)
    spiece = (src % SHR) // PR  # which AllGather piece holds the source row
    srel = spid - spiece * (nc_ * PR)  # row id within the piece's table
    dcore = dst // SHR
    dloc_all = dst % SHR

    # --- group edges by (core, dst block, src piece); dst-sorted per group
    per_core_blocks = [
        [[None] * NPIECE for _ in range(NB)] for _ in range(nc_)
    ]  # [core][block][piece] -> (src_rel, dst_local_in_block)
    nmax = np.zeros((NB, NPIECE), np.int64)
    for c in range(nc_):
        sel = np.nonzero(dcore == c)[0]
        dl = dloc_all[sel]
        blk = dl // P
        for b in range(NB):
            m = blk == b
            sb_, db_, pb_ = srel[sel[m]], dl[m] % P, spiece[sel[m]]
            for p in range(NPIECE):
                mp = pb_ == p
                s_b, d_b = sb_[mp], db_[mp]
                o = np.argsort(d_b, kind="stable")
                per_core_blocks[c][b][p] = (s_b[o], d_b[o])
                nmax[b, p] = max(nmax[b, p], mp.sum())

    # chunks per (block, piece), shared across cores; each piece gathered
    # separately so its chunk range starts on a chunk boundary
    CHP = np.maximum(1, -(-nmax // P)).astype(int)  # [NB, NPIECE]
    CH = CHP.sum(1)  # total chunks per block
    OFF = np.concatenate([[0], np.cumsum(CH)]).astype(int)
    TOTC = int(OFF[-1])
    CHMAX = int(CH.max())

    gidx = np.zeros((nc_, P, 8 * TOTC), np.int16)
    dlocs = np.full((nc_, P, TOTC), 1000.0, np.float16)
    # per-dst [start, end) slot ranges per (block, piece) (dst-sorted edges;
    # slot ids are global within the block)
    st = np.zeros((nc_, P, NPIECE * NB), np.float32)
    en = np.zeros((nc_, P, NPIECE * NB), np.float32)
    for c in range(nc_):
        for b in range(NB):
            c0 = 0
            for p in range(NPIECE):
                s_p, d_l = per_core_blocks[c][b][p]
                n = len(s_p)
                nid = CHP[b, p] * P
                idxs = np.zeros(nid, np.int64)
                idxs[:n] = s_p
                j = np.arange(nid)
                # idx block wrapped into 16 partitions, replicated across
                # the 8 GpSimd Q7 cores
                coloff = 8 * (OFF[b] + c0)
                for k in range(8):
                    gidx[c, 16 * k + (j % 16), coloff + j // 16] = idxs
                jr = np.arange(n)
                dlocs[c, jr % P, OFF[b] + c0 + jr // P] = d_l
                cnt = np.bincount(d_l, minlength=P)
                ends = np.cumsum(cnt)
                base = c0 * P
                st[c, :, p * NB + b] = base + ends - cnt
                en[c, :, p * NB + b] = base + ends
                c0 += CHP[b, p]

    # --- x shards, feature-major (KD, 128, SH) fp16
    xT = np.zeros((nc_, KD, P, SH), np.float16)
    for c in range(nc_):
        rows = x[c * SHR : min((c + 1) * SHR, N)]
        xp = np.zeros((SH, D), np.float32)
        xp[: len(rows)] = rows
        xT[c] = xp.T.reshape(KD, P, SH).astype(np.float16)

    # --- weights (fp16)
    W_all = np.zeros((L, KD, P, D), np.float16)
    WT_all = np.zeros((L, KD, P, D), np.float16)
    A_all = np.zeros((L, KD, P, 2 * H), np.float16)
    b_l, g_l, be_l = [], [], []
    for l in range(L):
        W = np.asarray(inputs[f"W{l}"], np.float32)
        W_all[l] = W.reshape(KD, P, D).astype(np.float16)
        WT_all[l] = np.ascontiguousarray(W.T).reshape(KD, P, D).astype(np.float16)
        A = np.zeros((D, 2 * H), np.float32)
        a_s = np.asarray(inputs[f"as{l}"], np.float32)
        a_d = np.asarray(inputs[f"ad{l}"], np.float32)
        for h in range(H):
            A[h * C : (h + 1) * C, h] = a_s[h]
            A[h * C : (h + 1) * C, H + h] = a_d[h]
        A_all[l] = A.reshape(KD, P, 2 * H).astype(np.float16)
        b_l.append(np.asarray(inputs[f"b{l}"], np.float32))
        g_l.append(np.asarray(inputs[f"g{l}"], np.float32))
        be_l.append(np.asarray(inputs[f"be{l}"], np.float32))

    skip_b = all(not b.any() for b in b_l)
    skip_g = all((g == 1.0).all() for g in g_l)
    skip_be = all(not be.any() for be in be_l)

    fc_W = np.asarray(inputs["fc_W"], np.float32).reshape(KD, P, OUT)
    fc_Wr = np.ascontiguousarray(fc_W.transpose(1, 0, 2)).astype(np.float16)
    fc_b = np.asarray(inputs["fc_b"], np.float32)
    skip_fcb = not fc_b.any()

    # --- pool masks (0/1 membership) fp16; counts precomputed on host
    poolmask = np.zeros((nc_, NB, P, G), np.float16)
    for c in range(nc_):
        lo = c * SHR
        hi = min((c + 1) * SHR, N)
        loc = np.arange(hi - lo)
        g_of = batch[lo:hi]
        poolmask[c, loc // P, loc % P, g_of] = 1.0
    counts = np.bincount(batch, minlength=G).astype(np.float32)
    crec = (1.0 / np.maximum(counts, 1.0)).reshape(G, 1)

    meta = dict(
        SH=SH, NB=NB, KD=KD, PR=PR, CH=CH, CHP=CHP, OFF=OFF, TOTC=TOTC,
        CHMAX=CHMAX,
        skip_b=skip_b, skip_g=skip_g, skip_be=skip_be, skip_fcb=skip_fcb,
    )

    in_maps = []
    for c in range(nc_):
        m = dict(
            xT=xT[c],
            W_all=W_all,
            WT_all=WT_all,
            A_all=A_all,
            fc_W=fc_Wr,
            gidx=gidx[c],
            dloc=dlocs[c],
            st=st[c],
            en=en[c],
            poolmask=poolmask[c],
            crec=crec,
        )
        if not skip_b:
            m["b_rep"] = np.broadcast_to(
                np.stack(b_l)[:, None, :], (L, P, D)
            ).copy()
        if not skip_g:
            m["g_rep"] = np.broadcast_to(
                np.stack(g_l)[:, None, :], (L, P, D)
            ).copy()
        if not skip_be:
            m["be_rep"] = np.broadcast_to(
                np.stack(be_l)[:, None, :], (L, P, D)
            ).copy()
        if not skip_fcb:
            m["fcb_rep"] = np.broadcast_to(fc_b[None, :], (P, OUT)).copy()
        in_maps.append(m)
    return in_maps, meta


# --------------------------------------------------------------------------
# Device program
# --------------------------------------------------------------------------


def build(tc, cfg, meta, I, out_ap):
    nc = tc.nc
    nc_cores = cfg["n_cores"]
    D, H, G, OUT, L = cfg["D"], cfg["H"], cfg["G"], cfg["OUT"], cfg["L"]
    NEG, EPS = cfg["NEG"], cfg["EPS"]
    NPIECE, NQ, ROWF = cfg["NPIECE"], cfg["NQ"], cfg["ROWF"]
    SH, NB, KD, PR = meta["SH"], meta["NB"], meta["KD"], meta["PR"]
    CH, OFF, TOTC, CHMAX = meta["CH"], meta["OFF"], meta["TOTC"], meta["CHMAX"]
    CHP = meta["CHP"]
    H2 = 2 * H
    BPP = NB // NPIECE  # blocks per AllGather piece

    rg = [list(range(nc_cores))]
    shared = "Shared" if nc_cores > 4 else "Local"

    from contextlib import ExitStack

    ctx = ExitStack()
    res = ctx.enter_context(tc.tile_pool(name="res", bufs=1))
    dram = ctx.enter_context(tc.tile_pool(name="dram", bufs=1, space="DRAM"))
    psum = ctx.enter_context(tc.tile_pool(name="psum", bufs=1, space="PSUM"))
    sb = ctx.enter_context(tc.tile_pool(name="sb", bufs=1))

    # ---------------- resident tiles
    xT_sb = [res.tile([P, SH], F16, name=f"xT{k}") for k in range(KD)]
    xn_sb = [res.tile([P, D], F16, name=f"xn{b}") for b in range(NB)]
    henm_sb = [res.tile([P, H2], F16, name=f"henm{b}") for b in range(NB)]
    W_sb = [res.tile([P, D], F16, name=f"W{k}") for k in range(KD)]
    WT_sb = [res.tile([P, D], F16, name=f"WT{k}") for k in range(KD)]
    A_sb = [res.tile([P, H2], F16, name=f"A{k}") for k in range(KD)]
    wa_sb = [res.tile([P, H2], F16, name=f"wa{k}") for k in range(KD)]
    idx_sb = res.tile([P, 8 * TOTC], I16, name="idx_sb")
    dloc_sb = res.tile([P, TOTC], F16, name="dloc_sb")
    st_sb = res.tile([P, NPIECE * NB], F32, name="st_sb")
    en_sb = res.tile([P, NPIECE * NB], F32, name="en_sb")
    id128 = res.tile([P, P], F16, name="id128")
    make_identity(nc, id128[:])
    # iota3[p, a, b] = b   (dst-id pattern for the mask equality compare)
    iota3 = res.tile([P, CHMAX, P], F16, name="iota3")
    nc.gpsimd.iota(
        iota3[:], pattern=[[0, CHMAX], [1, P]], base=0, channel_multiplier=0,
        allow_small_or_imprecise_dtypes=True,
    )
    # iotaG[p, a, b] = a*128 + b  (edge-slot id for the staircase compare)
    iotaG = res.tile([P, CHMAX, P], F32, name="iotaG")
    nc.gpsimd.iota(
        iotaG[:], pattern=[[P, CHMAX], [1, P]], base=0, channel_multiplier=0,
        allow_small_or_imprecise_dtypes=True,
    )

    b_rep = g_rep = be_rep = None
    if not meta["skip_b"]:
        b_rep = res.tile([P, D], F32, name="b_rep")
    if not meta["skip_g"]:
        g_rep = res.tile([P, D], F32, name="g_rep")
    if not meta["skip_be"]:
        be_rep = res.tile([P, D], F32, name="be_rep")

    nc.sync.dma_start(out=idx_sb[:], in_=I["gidx"][:])
    nc.sync.dma_start(out=dloc_sb[:], in_=I["dloc"][:])
    nc.sync.dma_start(out=st_sb[:], in_=I["st"][:])
    nc.sync.dma_start(out=en_sb[:], in_=I["en"][:])
    for k in range(KD):
        nc.sync.dma_start(out=xT_sb[k][:], in_=I["xT"][k])

    # ---------------- DRAM comm buffers
    ag_in = [
        [dram.tile([PR, ROWF], F16, name=f"ag_in{l}_{p}") for p in range(NPIECE)]
        for l in range(L)
    ]
    ag_out = [
        [
            dram.tile(
                [nc_cores * PR, ROWF], F16, name=f"ag_out{l}_{p}",
                addr_space=shared,
            )
            for p in range(NPIECE)
        ]
        for l in range(L)
    ]
    mkd = dram.tile([P, TOTC * P], F16, name="mkd")
    mkTd = dram.tile([P, TOTC * P], F16, name="mkTd")
    ar_in = dram.tile([G, D], F32, name="ar_in")
    ar_out = dram.tile([G, D], F32, name="ar_out", addr_space=shared)

    # ---------------- helpers
    def load_weights(l):
        for k in range(KD):
            nc.sync.dma_start(out=W_sb[k][:], in_=I["W_all"][l, k])
            nc.sync.dma_start(out=WT_sb[k][:], in_=I["WT_all"][l, k])
            nc.sync.dma_start(out=A_sb[k][:], in_=I["A_all"][l, k])
        if b_rep is not None:
            nc.sync.dma_start(out=b_rep[:], in_=I["b_rep"][l])
        if g_rep is not None:
            nc.sync.dma_start(out=g_rep[:], in_=I["g_rep"][l])
        if be_rep is not None:
            nc.sync.dma_start(out=be_rep[:], in_=I["be_rep"][l])

    def compute_wa():
        # wa = W @ A, feature-major chunks (KD x [128, 2H])
        for ic in range(KD):
            wa_ps = psum.tile([P, H2], F32, name="wa_ps", tag="ed", bufs=2)
            for oc in range(KD):
                nc.tensor.matmul(
                    out=wa_ps[:],
                    lhsT=WT_sb[oc][:, ic * P : (ic + 1) * P],
                    rhs=A_sb[oc][:],
                    start=(oc == 0),
                    stop=(oc == KD - 1),
                )
            nc.vector.tensor_copy(out=wa_sb[ic][:], in_=wa_ps[:])

    def gemm_block(l, b):
        # h (node-major) and e=[e_src|e_dst] for block b -> ag_in row piece
        bs = slice(b * P, (b + 1) * P)
        h_ps = psum.tile([P, D], F32, name="h_ps", tag="big", bufs=2)
        for k in range(KD):
            nc.tensor.matmul(
                out=h_ps[:], lhsT=xT_sb[k][:, bs], rhs=W_sb[k][:],
                start=(k == 0), stop=(k == KD - 1),
            )
        he_ps = psum.tile([P, H2], F32, name="he_ps", tag="ed", bufs=2)
        for k in range(KD):
            nc.tensor.matmul(
                out=he_ps[:], lhsT=xT_sb[k][:, bs], rhs=wa_sb[k][:],
                start=(k == 0), stop=(k == KD - 1),
            )
        nc.vector.tensor_copy(out=henm_sb[b][:], in_=he_ps[:])
        hrow = sb.tile([P, ROWF], F16, name="hrow", tag="hrow", bufs=3)
        nc.scalar.activation(hrow[:, 0:D], h_ps[:], ACT.Copy)
        nc.vector.tensor_copy(out=hrow[:, D : D + H], in_=he_ps[:, 0:H])
        nc.vector.memset(hrow[:, D + H : ROWF], 0.0)
        p = b // BPP
        rr = (b % BPP) * P
        nc.scalar.dma_start(out=ag_in[l][p][rr : rr + P, :], in_=hrow[:])

    def ag_piece(l, p):
        nc.gpsimd.collective_compute(
            "AllGather",
            ALU.bypass,
            replica_groups=rg,
            ins=[ag_in[l][p][:].opt()],
            outs=[ag_out[l][p][:].opt()],
        )

    def edge_block(l, b):
        chb = int(CH[b])
        off = int(OFF[b])
        gt = sb.tile([P, CHMAX, ROWF], F16, name="gt", tag="gt", bufs=3)
        # one gather per AllGather piece (each <= 1024 idxs, the SWDGE cap)
        c0 = 0
        for p in range(NPIECE):
            c1 = c0 + int(CHP[b, p])
            nidx = (c1 - c0) * P
            assert nidx <= 1024
            nc.gpsimd.dma_gather(
                out_ap=gt[:, c0:c1, :],
                in_ap=ag_out[l][p][:],
                idxs_ap=idx_sb[:, 8 * (off + c0) : 8 * (off + c1)],
                num_idxs=nidx,
                num_idxs_reg=nidx,
                elem_size=ROWF,
                queue_num=((NPIECE * b + p) % NQ),
            )
            c0 = c1
        gs = sb.tile([P, CHMAX, D + H], F16, name="gs", tag="gs", bufs=3)
        out_ps = psum.tile([P, D], F32, name="out_ps", tag="big", bufs=2)
        den_ps = psum.tile([P, H], F32, name="den_ps", tag="den", bufs=1)
        # masks for all chunks of the block, built on DVE:
        #   mk[jj, ch, d]  = (dloc[jj, ch] == d)
        #   mkT[d, ch, jj] = (st[d] <= ch*128+jj < en[d])   (edges dst-sorted)
        mk_all = sb.tile([P, CHMAX, P], F16, name="mk_all", tag="mk_all", bufs=3)
        mkT_all = sb.tile([P, CHMAX, P], F16, name="mkT_all", tag="mkT_all", bufs=3)
        mks = mk_all[:, 0:chb, :].rearrange("p a b -> p (a b)")
        mkTs = mkT_all[:, 0:chb, :].rearrange("p a b -> p (a b)")
        dsl = slice(off * P, (off + chb) * P)
        if l == 0:
            # masks are layer-invariant: build once on DVE, cache in DRAM
            nc.vector.tensor_tensor(
                out=mk_all[:, 0:chb, :],
                in0=iota3[:, 0:chb, :],
                in1=dloc_sb[:, off : off + chb].unsqueeze(2).to_broadcast(
                    [P, chb, P]
                ),
                op=ALU.is_equal,
            )
            c0 = 0
            for p in range(NPIECE):
                c1 = c0 + int(CHP[b, p])
                col = p * NB + b
                nc.vector.tensor_scalar(
                    out=mkT_all[:, c0:c1, :], in0=iotaG[:, c0:c1, :],
                    scalar1=st_sb[:, col : col + 1], scalar2=None,
                    op0=ALU.is_ge,
                )
                nc.vector.scalar_tensor_tensor(
                    out=mkT_all[:, c0:c1, :], in0=iotaG[:, c0:c1, :],
                    scalar=en_sb[:, col : col + 1], in1=mkT_all[:, c0:c1, :],
                    op0=ALU.is_lt, op1=ALU.mult,
                )
                c0 = c1
            nc.scalar.dma_start(out=mkd[:, dsl], in_=mks)
            nc.scalar.dma_start(out=mkTd[:, dsl], in_=mkTs)
        else:
            nc.scalar.dma_start(out=mks, in_=mkd[:, dsl])
            nc.scalar.dma_start(out=mkTs, in_=mkTd[:, dsl])
        # e_dst per edge slot via mask matmuls into one PSUM strip
        ed_all = psum.tile([P, CHMAX * H], F32, name="ed_all", tag="ed", bufs=2)
        for ch in range(chb):
            nc.tensor.matmul(
                out=ed_all[:, ch * H : (ch + 1) * H],
                lhsT=mkT_all[:, ch, :], rhs=henm_sb[b][:, H:H2],
                start=True, stop=True, skip_group_check=True,
            )
        e_all = sb.tile([P, CHMAX, H], F16, name="e_all", tag="e_all", bufs=3)
        nc.vector.tensor_tensor(
            out=e_all[:, 0:chb, :], in0=gt[:, 0:chb, D : D + H],
            in1=ed_all[:, 0 : chb * H].rearrange("p (a h) -> p a h", h=H),
            op=ALU.add,
        )
        pr_all = sb.tile([P, CHMAX, H], F16, name="pr_all", tag="pr_all", bufs=3)
        nc.scalar.activation(
            pr_all[:, 0:chb, :], e_all[:, 0:chb, :], ACT.Prelu, alpha=NEG
        )
        nc.scalar.activation(gs[:, 0:chb, D : D + H], pr_all[:, 0:chb, :], ACT.Exp)
        nc.vector.tensor_tensor(
            out=gs[:, 0:chb, 0:D].rearrange("p a (h c) -> p a h c", h=H),
            in0=gt[:, 0:chb, 0:D].rearrange("p a (h c) -> p a h c", h=H),
            in1=gs[:, 0:chb, D : D + H].unsqueeze(3).to_broadcast(
                [P, chb, H, D // H]
            ),
            op=ALU.mult,
        )
        for ch in range(chb):
            nc.tensor.matmul(
                out=den_ps[:], lhsT=mk_all[:, ch, :], rhs=gs[:, ch, D : D + H],
                start=(ch == 0), stop=(ch == chb - 1),
            )
        for ch in range(chb):
            nc.tensor.matmul(
                out=out_ps[:], lhsT=mk_all[:, ch, :], rhs=gs[:, ch, 0:D],
                start=(ch == 0), stop=(ch == chb - 1),
            )

        # ----- block epilogue: normalize by segment softmax denom, LN, relu
        den_sb = sb.tile([P, H], F32, name="den_sb", tag="den_sb", bufs=2)
        nc.vector.tensor_scalar_add(out=den_sb[:], in0=den_ps[:], scalar1=1e-16)
        rec = sb.tile([P, H], F32, name="rec", tag="rec", bufs=2)
        nc.vector.reciprocal(out=rec[:], in_=den_sb[:])
        y_sb = sb.tile([P, D], F32, name="y_sb", tag="y_sb", bufs=2)
        nc.vector.tensor_tensor(
            out=y_sb[:].rearrange("p (h c) -> p h c", h=H),
            in0=out_ps[:].rearrange("p (h c) -> p h c", h=H),
            in1=rec[:].unsqueeze(2).to_broadcast([P, H, D // H]),
            op=ALU.mult,
        )
        if b_rep is not None:
            nc.vector.tensor_add(out=y_sb[:], in0=y_sb[:], in1=b_rep[:])
        # mean and variance sums on the ACT engine (it has spare capacity)
        sq16 = sb.tile([P, D], F16, name="sq16", tag="sq16", bufs=2)
        sy = sb.tile([P, 1], F32, name="sy", tag="sy", bufs=2)
        nc.scalar.activation(sq16[:], y_sb[:], ACT.Copy, accum_out=sy[:, 0:1])
        nmu = sb.tile([P, 1], F32, name="nmu", tag="nmu", bufs=2)
        nc.scalar.mul(nmu[:], sy[:], -1.0 / D)  # nmu = -mu
        ssq = sb.tile([P, 1], F32, name="ssq", tag="ssq", bufs=2)
        nc.scalar.activation(
            sq16[:], y_sb[:], ACT.Square, bias=nmu[:, 0:1],
            accum_out=ssq[:, 0:1],
        )
        # rstd = 1/sqrt(ssq/D + eps) on DVE via bit-trick + 2 Newton steps
        # (avoids Sqrt/Ln on the ACT engine, whose tables clash with Exp)
        vv = sb.tile([P, 1], F32, name="vv", tag="vv", bufs=2)
        nc.vector.tensor_scalar(
            out=vv[:], in0=ssq[:], scalar1=1.0 / D, scalar2=float(EPS),
            op0=ALU.mult, op1=ALU.add,
        )
        ri = sb.tile([P, 1], mybir.dt.int32, name="ri", tag="ri", bufs=2)
        nc.vector.tensor_scalar(
            out=ri[:], in0=vv[:].bitcast(mybir.dt.int32), scalar1=1,
            scalar2=-1, op0=ALU.logical_shift_right, op1=ALU.bitwise_xor,
        )
        nc.vector.tensor_scalar_add(out=ri[:], in0=ri[:], scalar1=0x5F3759DF + 1)
        rstd = sb.tile([P, 1], F32, name="rstd", tag="rstd", bufs=2)
        nc.vector.tensor_copy(out=rstd[:], in_=ri[:].bitcast(F32))
        for _ in range(2):  # 2 Newton steps: rel err ~5e-6
            nr_a = sb.tile([P, 1], F32, name="nr_a", tag="nr_a", bufs=2)
            nc.vector.tensor_mul(out=nr_a[:], in0=rstd[:], in1=rstd[:])
            nc.vector.tensor_mul(out=nr_a[:], in0=nr_a[:], in1=vv[:])
            nc.vector.tensor_scalar(
                out=nr_a[:], in0=nr_a[:], scalar1=-0.5, scalar2=1.5,
                op0=ALU.mult, op1=ALU.add,
            )
            nc.vector.tensor_mul(out=rstd[:], in0=rstd[:], in1=nr_a[:])
        mm = sb.tile([P, 1], F32, name="mm", tag="mm", bufs=2)
        nc.vector.tensor_mul(out=mm[:], in0=nmu[:], in1=rstd[:])
        if g_rep is None and be_rep is None:
            nc.scalar.activation(
                xn_sb[b][:], y_sb[:], ACT.Relu,
                scale=rstd[:, 0:1], bias=mm[:, 0:1],
            )
        else:
            ln_sb = sb.tile([P, D], F32, name="ln_sb", tag="ln_sb", bufs=2)
            nc.scalar.activation(
                ln_sb[:], y_sb[:], ACT.Identity,
                scale=rstd[:, 0:1], bias=mm[:, 0:1],
            )
            if g_rep is not None:
                nc.vector.tensor_mul(out=ln_sb[:], in0=ln_sb[:], in1=g_rep[:])
            if be_rep is not None:
                nc.vector.tensor_add(out=ln_sb[:], in0=ln_sb[:], in1=be_rep[:])
            nc.scalar.activation(xn_sb[b][:], ln_sb[:], ACT.Relu)

    def trans_block(b):
        # xn block -> feature-major xT for the next layer's GEMM
        for k in range(KD):
            t_ps = psum.tile([P, P], F16, name="t_ps", tag="tr", bufs=2)
            nc.tensor.transpose(
                out=t_ps[:], in_=xn_sb[b][:, k * P : (k + 1) * P],
                identity=id128[:],
            )
            nc.vector.tensor_copy(
                out=xT_sb[k][:, b * P : (b + 1) * P], in_=t_ps[:]
            )

    # ---------------- program
    load_weights(0)
    compute_wa()
    for b in range(NB):
        gemm_block(0, b)
        if (b + 1) % BPP == 0:
            ag_piece(0, (b + 1) // BPP - 1)

    pm_pool = ctx.enter_context(tc.tile_pool(name="pm", bufs=2))
    pool_ps = psum.tile([G, D], F32, name="pool_ps", tag="pool", bufs=1)

    def pool_block(b):
        pm_sb = pm_pool.tile([P, G], F16, name="pm_sb", tag="pm_sb", bufs=2)
        nc.scalar.dma_start(out=pm_sb[:], in_=I["poolmask"][b])
        nc.tensor.matmul(
            out=pool_ps[:], lhsT=pm_sb[:], rhs=xn_sb[b][:],
            start=(b == 0), stop=(b == NB - 1),
        )

    for l in range(L):
        if l + 1 < L:
            load_weights(l + 1)
            compute_wa()
        for b in range(NB):
            edge_block(l, b)
            if l + 1 < L:
                trans_block(b)
                gemm_block(l + 1, b)
                if (b + 1) % BPP == 0:
                    ag_piece(l + 1, (b + 1) // BPP - 1)
            else:
                pool_block(b)

    # ---------------- pooling epilogue (counts precomputed on host) + FC
    pool_sb = res.tile([G, D], F32, name="pool_sb")
    nc.vector.tensor_copy(out=pool_sb[:], in_=pool_ps[:])
    nc.sync.dma_start(out=ar_in[:], in_=pool_sb[:])
    nc.gpsimd.collective_compute(
        "AllReduce",
        ALU.add,
        replica_groups=rg,
        ins=[ar_in[:].opt()],
        outs=[ar_out[:].opt()],
    )
    pf_sb = res.tile([G, D], F32, name="pf_sb")
    nc.sync.dma_start(out=pf_sb[:], in_=ar_out[:])
    crec_sb = res.tile([G, 1], F32, name="crec_sb")
    nc.sync.dma_start(out=crec_sb[:], in_=I["crec"][:])
    pn16 = res.tile([G, D], F16, name="pn16")
    nc.vector.tensor_tensor(
        out=pn16[:], in0=pf_sb[:],
        in1=crec_sb[:].to_broadcast([G, D]), op=ALU.mult,
    )
    # transpose pooled -> (KD chunks of (128, G))
    pT_sb = res.tile([P, KD, G], F16, name="pT_sb")
    for k in range(KD):
        t2_ps = psum.tile([P, G], F16, name="t2_ps", tag="tr", bufs=2)
        nc.tensor.transpose(
            out=t2_ps[:], in_=pn16[:, k * P : (k + 1) * P], identity=id128[:]
        )
        nc.vector.tensor_copy(out=pT_sb[:, k, :], in_=t2_ps[:])
    fcw_sb = res.tile([P, KD, OUT], F16, name="fcw_sb")
    nc.sync.dma_start(out=fcw_sb[:], in_=I["fc_W"][:])
    fc_ps = psum.tile([G, OUT], F32, name="fc_ps", tag="big", bufs=2)
    for k in range(KD):
        nc.tensor.matmul(
            out=fc_ps[:], lhsT=pT_sb[:, k, :], rhs=fcw_sb[:, k, :],
            start=(k == 0), stop=(k == KD - 1),
        )
    o_sb = res.tile([G, OUT], F32, name="o_sb")
    if not meta["skip_fcb"]:
        fcb_rep = res.tile([P, OUT], F32, name="fcb_rep")
        nc.sync.dma_start(out=fcb_rep[:], in_=I["fcb_rep"][:])
        nc.vector.tensor_add(out=o_sb[:], in0=fc_ps[:], in1=fcb_rep[0:G, :])
    else:
        nc.vector.tensor_copy(out=o_sb[:], in_=fc_ps[:])
    nc.sync.dma_start(out=out_ap[:], in_=o_sb[:])
    ctx.close()


# --------------------------------------------------------------------------
# Entry point
# --------------------------------------------------------------------------


def kernel(**inputs):
    global LAST_RESULTS
    cfg = _full_cfg()
    in_maps, meta = _prep(inputs, cfg)

    nc = bacc.Bacc(
        "TRN2",
        target_bir_lowering=False,
        debug=False,
        enable_asserts=False,
        num_devices=cfg["n_cores"],
        num_swdge_queues=cfg["NQ"],
    )
    I = {}
    for name, arr in in_maps[0].items():
        I[name] = nc.dram_tensor(
            name, arr.shape, mybir.dt.from_np(arr.dtype), kind="ExternalInput"
        ).ap()
    out_ap = nc.dram_tensor(
        "out", (cfg["G"], cfg["OUT"]), F32, kind="ExternalOutput"
    ).ap()

    with tile.TileContext(nc) as tc:
        build(tc, cfg, meta, I, out_ap)
    nc.compile()

    trace = bool(int(os.environ.get("GAT_TRACE", "0")))
    res = run_bass_kernel_spmd(
        nc,
        in_maps,
        core_ids=list(range(cfg["n_cores"])),
        trace=trace,
    )
    LAST_RESULTS = res
    return np.asarray(res.results[0]["out"])


# revision 31
# speedup vs baseline: 1.3286x; 1.0571x over previous
"""GAT (3-layer, 8-head) forward on 8 Trainium2 NeuronCores — v2.

Strategy (graph/data parallel, per sharding hint):
  - Nodes sharded across 8 cores (2500 -> padded 2560 rows each).
  - All tensor data on device is fp16 (PSUM accumulation fp32): 4x faster
    PE matmuls vs fp32, 2x DVE, half the DMA/collective traffic.
  - Per layer, per 128-node block: h = x @ W and e = x @ (W A) via
    feature-major GEMMs; [h | e_src] rows (1152B, 256B-aligned) DMA'd to a
    DRAM shard and AllGather'd in NPIECE pieces (piece-major row ids) so the
    collective overlaps the previous layer's edge phase.
  - Edge phase per dst block: ONE batched dma_gather (int16 idxs) pulls all
    of the block's source rows (amortizes the ~1us SWDGE fixed cost, spreads
    across 4 SWDGE queues); 0/1 scatter masks are built on-chip from dst-id
    compares (iota vs per-chunk dst column) instead of DMA'd from the host;
    segment reduction of [ee*h | ee] happens on the TensorEngine into PSUM.
  - Softmax normalization post-aggregation (exact up to fp reassociation;
    |e| is O(1) so no max-subtraction needed).
  - LayerNorm epilogue avoids Sqrt (its ACT table set clashes with Exp):
    rstd = exp(-0.5*ln(var+eps)); all ACT functions used (Exp, Prelu, Ln,
    Identity, Relu, Copy) live in one table set -> no ACT_TABLE_LOAD thrash.
  - Next layer's GEMM + AllGather pieces are emitted interleaved with the
    edge loop so the collectives hide behind compute.
  - Mean-pool by graph via host-built 0/1 masks + AllReduce, then FC head.
"""

import os
import sys

sys.path.insert(0, "/opt/trn_rl_repo")

import numpy as np

import concourse.bass as bass
import concourse.mybir as mybir
import concourse.tile as tile
from concourse import bacc
from concourse.bass_utils import run_bass_kernel_spmd
from concourse.masks import make_identity

F32 = mybir.dt.float32
F16 = mybir.dt.float16
I16 = mybir.dt.int16
ALU = mybir.AluOpType
ACT = mybir.ActivationFunctionType

P = 128

# Results of the last run (for test harnesses).
LAST_RESULTS = None


def _full_cfg():
    return dict(
        n_cores=8,
        N=20000,
        D=512,
        H=8,
        G=128,
        OUT=128,
        NEG=0.2,
        EPS=1e-5,
        L=3,
        NPIECE=2,
        NQ=4,  # SWDGE queues for the gathers
        ROWF=640,  # padded gather row (fp16 elems); 640*2 = 1280B, 256B-aligned
    )


# --------------------------------------------------------------------------
# Host-side preprocessing: pure index manipulation / relayout (dtype casts
# and zero padding only).
# --------------------------------------------------------------------------


def _prep(inputs, cfg):
    nc_ = cfg["n_cores"]
    N, D, H, G, OUT, L = cfg["N"], cfg["D"], cfg["H"], cfg["G"], cfg["OUT"], cfg["L"]
    NPIECE, ROWF = cfg["NPIECE"], cfg["ROWF"]
    C = D // H

    x = np.asarray(inputs["x"], np.float32)
    ei = np.asarray(inputs["edge_index"])
    batch = np.asarray(inputs["batch"]).astype(np.int64)

    SHR = (N + nc_ - 1) // nc_  # real nodes per core
    SH = ((SHR + P - 1) // P) * P  # padded nodes per core
    NB = SH // P
    KD = D // P
    PR = SH // NPIECE  # rows per AllGather piece

    # --- edges with self loops
    loops = np.arange(N, dtype=np.int64)
    src = np.concatenate([ei[0], loops])
    dst = np.concatenate([ei[1], loops])

    # piece-major global row id of a node (matches the piecewise AllGather)
    def pid_of(n):
        c = n // SHR
        r = n % SHR
        p = r // PR
        return p * (nc_ * PR) + c * PR + (r % PR)

    spid = pid_of(src)
    spiece = (src % SHR) // PR  # which AllGather piece holds the source row
    srel = spid - spiece * (nc_ * PR)  # row id within the piece's table
    dcore = dst // SHR
    dloc_all = dst % SHR

    # --- group edges by (core, dst block, src piece); dst-sorted per group
    per_core_blocks = [
        [[None] * NPIECE for _ in range(NB)] for _ in range(nc_)
    ]  # [core][block][piece] -> (src_rel, dst_local_in_block)
    nmax = np.zeros((NB, NPIECE), np.int64)
    for c in range(nc_):
        sel = np.nonzero(dcore == c)[0]
        dl = dloc_all[sel]
        blk = dl // P
        for b in range(NB):
            m = blk == b
            sb_, db_, pb_ = srel[sel[m]], dl[m] % P, spiece[sel[m]]
            for p in range(NPIECE):
                mp = pb_ == p
                s_b, d_b = sb_[mp], db_[mp]
                o = np.argsort(d_b, kind="stable")
                per_core_blocks[c][b][p] = (s_b[o], d_b[o])
                nmax[b, p] = max(nmax[b, p], mp.sum())

    # chunks per (block, piece), shared across cores; each piece gathered
    # separately so its chunk range starts on a chunk boundary
    CHP = np.maximum(1, -(-nmax // P)).astype(int)  # [NB, NPIECE]
    CH = CHP.sum(1)  # total chunks per block
    OFF = np.concatenate([[0], np.cumsum(CH)]).astype(int)
    TOTC = int(OFF[-1])
    CHMAX = int(CH.max())

    gidx = np.zeros((nc_, P, 8 * TOTC), np.int16)
    dlocs = np.full((nc_, P, TOTC), 1000.0, np.float16)
    # per-dst [start, end) slot ranges per (block, piece) (dst-sorted edges;
    # slot ids are global within the block)
    st = np.zeros((nc_, P, NPIECE * NB), np.float32)
    en = np.zeros((nc_, P, NPIECE * NB), np.float32)
    for c in range(nc_):
        for b in range(NB):
            c0 = 0
            for p in range(NPIECE):
                s_p, d_l = per_core_blocks[c][b][p]
                n = len(s_p)
                nid = CHP[b, p] * P
                idxs = np.zeros(nid, np.int64)
                idxs[:n] = s_p
                j = np.arange(nid)
                # idx block wrapped into 16 partitions, replicated across
                # the 8 GpSimd Q7 cores
                coloff = 8 * (OFF[b] + c0)
                for k in range(8):
                    gidx[c, 16 * k + (j % 16), coloff + j // 16] = idxs
                jr = np.arange(n)
                dlocs[c, jr % P, OFF[b] + c0 + jr // P] = d_l
                cnt = np.bincount(d_l, minlength=P)
                ends = np.cumsum(cnt)
                base = c0 * P
                st[c, :, p * NB + b] = base + ends - cnt
                en[c, :, p * NB + b] = base + ends
                c0 += CHP[b, p]

    # --- x shards, feature-major (KD, 128, SH) fp16
    xT = np.zeros((nc_, KD, P, SH), np.float16)
    for c in range(nc_):
        rows = x[c * SHR : min((c + 1) * SHR, N)]
        xp = np.zeros((SH, D), np.float32)
        xp[: len(rows)] = rows
        xT[c] = xp.T.reshape(KD, P, SH).astype(np.float16)

    # --- weights (fp16)
    W_all = np.zeros((L, KD, P, D), np.float16)
    WT_all = np.zeros((L, KD, P, D), np.float16)
    A_all = np.zeros((L, KD, P, 2 * H), np.float16)
    b_l, g_l, be_l = [], [], []
    for l in range(L):
        W = np.asarray(inputs[f"W{l}"], np.float32)
        W_all[l] = W.reshape(KD, P, D).astype(np.float16)
        WT_all[l] = np.ascontiguousarray(W.T).reshape(KD, P, D).astype(np.float16)
        A = np.zeros((D, 2 * H), np.float32)
        a_s = np.asarray(inputs[f"as{l}"], np.float32)
        a_d = np.asarray(inputs[f"ad{l}"], np.float32)
        for h in range(H):
            A[h * C : (h + 1) * C, h] = a_s[h]
            A[h * C : (h + 1) * C, H + h] = a_d[h]
        A_all[l] = A.reshape(KD, P, 2 * H).astype(np.float16)
        b_l.append(np.asarray(inputs[f"b{l}"], np.float32))
        g_l.append(np.asarray(inputs[f"g{l}"], np.float32))
        be_l.append(np.asarray(inputs[f"be{l}"], np.float32))

    skip_b = all(not b.any() for b in b_l)
    skip_g = all((g == 1.0).all() for g in g_l)
    skip_be = all(not be.any() for be in be_l)

    fc_W = np.asarray(inputs["fc_W"], np.float32).reshape(KD, P, OUT)
    fc_Wr = np.ascontiguousarray(fc_W.transpose(1, 0, 2)).astype(np.float16)
    fc_b = np.asarray(inputs["fc_b"], np.float32)
    skip_fcb = not fc_b.any()

    # --- pool masks (0/1 membership) fp16; counts precomputed on host
    poolmask = np.zeros((nc_, NB, P, G), np.float16)
    for c in range(nc_):
        lo = c * SHR
        hi = min((c + 1) * SHR, N)
        loc = np.arange(hi - lo)
        g_of = batch[lo:hi]
        poolmask[c, loc // P, loc % P, g_of] = 1.0
    counts = np.bincount(batch, minlength=G).astype(np.float32)
    crec = (1.0 / np.maximum(counts, 1.0)).reshape(G, 1)

    meta = dict(
        SH=SH, NB=NB, KD=KD, PR=PR, CH=CH, CHP=CHP, OFF=OFF, TOTC=TOTC,
        CHMAX=CHMAX,
        skip_b=skip_b, skip_g=skip_g, skip_be=skip_be, skip_fcb=skip_fcb,
    )

    in_maps = []
    for c in range(nc_):
        m = dict(
            xT=xT[c],
            W_all=W_all,
            WT_all=WT_all,
            A_all=A_all,
            fc_W=fc_Wr,
            gidx=gidx[c],
            dloc=dlocs[c],
            st=st[c],
            en=en[c],
            poolmask=poolmask[c],
            crec=crec,
        )
        if not skip_b:
            m["b_rep"] = np.broadcast_to(
                np.stack(b_l)[:, None, :], (L, P, D)
            ).copy()
        if not skip_g:
            m["g_rep"] = np.broadcast_to(
                np.stack(g_l)[:, None, :], (L, P, D)
            ).copy()
        if not skip_be:
            m["be_rep"] = np.broadcast_to(
                np.stack(be_l)[:, None, :], (L, P, D)
            ).copy()
        if not skip_fcb:
            m["fcb_rep"] = np.broadcast_to(fc_b[None, :], (P, OUT)).copy()
        in_maps.append(m)
    return in_maps, meta


# --------------------------------------------------------------------------
# Device program
# --------------------------------------------------------------------------


def build(tc, cfg, meta, I, out_ap):
    nc = tc.nc
    nc_cores = cfg["n_cores"]
    D, H, G, OUT, L = cfg["D"], cfg["H"], cfg["G"], cfg["OUT"], cfg["L"]
    NEG, EPS = cfg["NEG"], cfg["EPS"]
    NPIECE, NQ, ROWF = cfg["NPIECE"], cfg["NQ"], cfg["ROWF"]
    SH, NB, KD, PR = meta["SH"], meta["NB"], meta["KD"], meta["PR"]
    CH, OFF, TOTC, CHMAX = meta["CH"], meta["OFF"], meta["TOTC"], meta["CHMAX"]
    CHP = meta["CHP"]
    H2 = 2 * H
    BPP = NB // NPIECE  # blocks per AllGather piece

    rg = [list(range(nc_cores))]
    shared = "Shared" if nc_cores > 4 else "Local"

    from contextlib import ExitStack

    ctx = ExitStack()
    res = ctx.enter_context(tc.tile_pool(name="res", bufs=1))
    dram = ctx.enter_context(tc.tile_pool(name="dram", bufs=1, space="DRAM"))
    psum = ctx.enter_context(tc.tile_pool(name="psum", bufs=1, space="PSUM"))
    sb = ctx.enter_context(tc.tile_pool(name="sb", bufs=1))

    # ---------------- resident tiles
    xT_sb = [res.tile([P, SH], F16, name=f"xT{k}") for k in range(KD)]
    xn_sb = [res.tile([P, D], F16, name=f"xn{b}") for b in range(NB)]
    henm_sb = [res.tile([P, H2], F16, name=f"henm{b}") for b in range(NB)]
    W_sb = [res.tile([P, D], F16, name=f"W{k}") for k in range(KD)]
    WT_sb = [res.tile([P, D], F16, name=f"WT{k}") for k in range(KD)]
    A_sb = [res.tile([P, H2], F16, name=f"A{k}") for k in range(KD)]
    wa_sb = [res.tile([P, H2], F16, name=f"wa{k}") for k in range(KD)]
    idx_sb = res.tile([P, 8 * TOTC], I16, name="idx_sb")
    dloc_sb = res.tile([P, TOTC], F16, name="dloc_sb")
    st_sb = res.tile([P, NPIECE * NB], F32, name="st_sb")
    en_sb = res.tile([P, NPIECE * NB], F32, name="en_sb")
    id128 = res.tile([P, P], F16, name="id128")
    make_identity(nc, id128[:])
    # iota3[p, a, b] = b   (dst-id pattern for the mask equality compare)
    iota3 = res.tile([P, CHMAX, P], F16, name="iota3")
    nc.gpsimd.iota(
        iota3[:], pattern=[[0, CHMAX], [1, P]], base=0, channel_multiplier=0,
        allow_small_or_imprecise_dtypes=True,
    )
    # iotaG[p, a, b] = a*128 + b  (edge-slot id for the staircase compare)
    iotaG = res.tile([P, CHMAX, P], F32, name="iotaG")
    nc.gpsimd.iota(
        iotaG[:], pattern=[[P, CHMAX], [1, P]], base=0, channel_multiplier=0,
        allow_small_or_imprecise_dtypes=True,
    )

    b_rep = g_rep = be_rep = None
    if not meta["skip_b"]:
        b_rep = res.tile([P, D], F32, name="b_rep")
    if not meta["skip_g"]:
        g_rep = res.tile([P, D], F32, name="g_rep")
    if not meta["skip_be"]:
        be_rep = res.tile([P, D], F32, name="be_rep")

    nc.sync.dma_start(out=idx_sb[:], in_=I["gidx"][:])
    nc.sync.dma_start(out=dloc_sb[:], in_=I["dloc"][:])
    nc.sync.dma_start(out=st_sb[:], in_=I["st"][:])
    nc.sync.dma_start(out=en_sb[:], in_=I["en"][:])
    for k in range(KD):
        nc.sync.dma_start(out=xT_sb[k][:], in_=I["xT"][k])

    # ---------------- DRAM comm buffers
    ag_in = [
        [dram.tile([PR, ROWF], F16, name=f"ag_in{l}_{p}") for p in range(NPIECE)]
        for l in range(L)
    ]
    ag_out = [
        [
            dram.tile(
                [nc_cores * PR, ROWF], F16, name=f"ag_out{l}_{p}",
                addr_space=shared,
            )
            for p in range(NPIECE)
        ]
        for l in range(L)
    ]
    ar_in = dram.tile([G, D], F32, name="ar_in")
    ar_out = dram.tile([G, D], F32, name="ar_out", addr_space=shared)

    # ---------------- helpers
    def load_weights(l):
        for k in range(KD):
            nc.sync.dma_start(out=W_sb[k][:], in_=I["W_all"][l, k])
            nc.sync.dma_start(out=WT_sb[k][:], in_=I["WT_all"][l, k])
            nc.sync.dma_start(out=A_sb[k][:], in_=I["A_all"][l, k])
        if b_rep is not None:
            nc.sync.dma_start(out=b_rep[:], in_=I["b_rep"][l])
        if g_rep is not None:
            nc.sync.dma_start(out=g_rep[:], in_=I["g_rep"][l])
        if be_rep is not None:
            nc.sync.dma_start(out=be_rep[:], in_=I["be_rep"][l])

    def compute_wa():
        # wa = W @ A, feature-major chunks (KD x [128, 2H])
        for ic in range(KD):
            wa_ps = psum.tile([P, H2], F32, name="wa_ps", tag="ed", bufs=2)
            for oc in range(KD):
                nc.tensor.matmul(
                    out=wa_ps[:],
                    lhsT=WT_sb[oc][:, ic * P : (ic + 1) * P],
                    rhs=A_sb[oc][:],
                    start=(oc == 0),
                    stop=(oc == KD - 1),
                )
            nc.vector.tensor_copy(out=wa_sb[ic][:], in_=wa_ps[:])

    def gemm_block(l, b):
        # h (node-major) and e=[e_src|e_dst] for block b -> ag_in row piece
        bs = slice(b * P, (b + 1) * P)
        h_ps = psum.tile([P, D], F32, name="h_ps", tag="big", bufs=2)
        for k in range(KD):
            nc.tensor.matmul(
                out=h_ps[:], lhsT=xT_sb[k][:, bs], rhs=W_sb[k][:],
                start=(k == 0), stop=(k == KD - 1),
            )
        he_ps = psum.tile([P, H2], F32, name="he_ps", tag="ed", bufs=2)
        for k in range(KD):
            nc.tensor.matmul(
                out=he_ps[:], lhsT=xT_sb[k][:, bs], rhs=wa_sb[k][:],
                start=(k == 0), stop=(k == KD - 1),
            )
        nc.vector.tensor_copy(out=henm_sb[b][:], in_=he_ps[:])
        hrow = sb.tile([P, ROWF], F16, name="hrow", tag="hrow", bufs=3)
        nc.scalar.activation(hrow[:, 0:D], h_ps[:], ACT.Copy)
        nc.vector.tensor_copy(out=hrow[:, D : D + H], in_=he_ps[:, 0:H])
        nc.vector.memset(hrow[:, D + H : ROWF], 0.0)
        p = b // BPP
        rr = (b % BPP) * P
        nc.sync.dma_start(out=ag_in[l][p][rr : rr + P, :], in_=hrow[:])

    def ag_piece(l, p):
        nc.gpsimd.collective_compute(
            "AllGather",
            ALU.bypass,
            replica_groups=rg,
            ins=[ag_in[l][p][:].opt()],
            outs=[ag_out[l][p][:].opt()],
        )

    def edge_block(l, b):
        chb = int(CH[b])
        off = int(OFF[b])
        gt = sb.tile([P, CHMAX, ROWF], F16, name="gt", tag="gt", bufs=3)
        # one gather per AllGather piece (each <= 1024 idxs, the SWDGE cap)
        c0 = 0
        for p in range(NPIECE):
            c1 = c0 + int(CHP[b, p])
            nidx = (c1 - c0) * P
            assert nidx <= 1024
            nc.gpsimd.dma_gather(
                out_ap=gt[:, c0:c1, :],
                in_ap=ag_out[l][p][:],
                idxs_ap=idx_sb[:, 8 * (off + c0) : 8 * (off + c1)],
                num_idxs=nidx,
                num_idxs_reg=nidx,
                elem_size=ROWF,
                queue_num=((NPIECE * b + p) % NQ),
            )
            c0 = c1
        gs = sb.tile([P, CHMAX, D + H], F16, name="gs", tag="gs", bufs=3)
        out_ps = psum.tile([P, D], F32, name="out_ps", tag="big", bufs=2)
        den_ps = psum.tile([P, H], F32, name="den_ps", tag="den", bufs=1)
        # masks for all chunks of the block, built on DVE:
        #   mk[jj, ch, d]  = (dloc[jj, ch] == d)
        #   mkT[d, ch, jj] = (st[d] <= ch*128+jj < en[d])   (edges dst-sorted)
        mk_all = sb.tile([P, CHMAX, P], F16, name="mk_all", tag="mk_all", bufs=3)
        nc.vector.tensor_tensor(
            out=mk_all[:, 0:chb, :],
            in0=iota3[:, 0:chb, :],
            in1=dloc_sb[:, off : off + chb].unsqueeze(2).to_broadcast(
                [P, chb, P]
            ),
            op=ALU.is_equal,
        )
        mkT_all = sb.tile([P, CHMAX, P], F16, name="mkT_all", tag="mkT_all", bufs=3)
        c0 = 0
        for p in range(NPIECE):
            c1 = c0 + int(CHP[b, p])
            col = p * NB + b
            nc.vector.tensor_scalar(
                out=mkT_all[:, c0:c1, :], in0=iotaG[:, c0:c1, :],
                scalar1=st_sb[:, col : col + 1], scalar2=None, op0=ALU.is_ge,
            )
            nc.vector.scalar_tensor_tensor(
                out=mkT_all[:, c0:c1, :], in0=iotaG[:, c0:c1, :],
                scalar=en_sb[:, col : col + 1], in1=mkT_all[:, c0:c1, :],
                op0=ALU.is_lt, op1=ALU.mult,
            )
            c0 = c1
        # e_dst per edge slot via mask matmuls into one PSUM strip
        ed_all = psum.tile([P, CHMAX * H], F32, name="ed_all", tag="ed", bufs=2)
        for ch in range(chb):
            nc.tensor.matmul(
                out=ed_all[:, ch * H : (ch + 1) * H],
                lhsT=mkT_all[:, ch, :], rhs=henm_sb[b][:, H:H2],
                start=True, stop=True, skip_group_check=True,
            )
        e_all = sb.tile([P, CHMAX, H], F16, name="e_all", tag="e_all", bufs=3)
        nc.vector.tensor_tensor(
            out=e_all[:, 0:chb, :], in0=gt[:, 0:chb, D : D + H],
            in1=ed_all[:, 0 : chb * H].rearrange("p (a h) -> p a h", h=H),
            op=ALU.add,
        )
        pr_all = sb.tile([P, CHMAX, H], F16, name="pr_all", tag="pr_all", bufs=3)
        nc.scalar.activation(
            pr_all[:, 0:chb, :], e_all[:, 0:chb, :], ACT.Prelu, alpha=NEG
        )
        nc.scalar.activation(gs[:, 0:chb, D : D + H], pr_all[:, 0:chb, :], ACT.Exp)
        nc.vector.tensor_tensor(
            out=gs[:, 0:chb, 0:D].rearrange("p a (h c) -> p a h c", h=H),
            in0=gt[:, 0:chb, 0:D].rearrange("p a (h c) -> p a h c", h=H),
            in1=gs[:, 0:chb, D : D + H].unsqueeze(3).to_broadcast(
                [P, chb, H, D // H]
            ),
            op=ALU.mult,
        )
        for ch in range(chb):
            nc.tensor.matmul(
                out=den_ps[:], lhsT=mk_all[:, ch, :], rhs=gs[:, ch, D : D + H],
                start=(ch == 0), stop=(ch == chb - 1),
            )
        for ch in range(chb):
            nc.tensor.matmul(
                out=out_ps[:], lhsT=mk_all[:, ch, :], rhs=gs[:, ch, 0:D],
                start=(ch == 0), stop=(ch == chb - 1),
            )

        # ----- block epilogue: normalize by segment softmax denom, LN, relu
        den_sb = sb.tile([P, H], F32, name="den_sb", tag="den_sb", bufs=2)
        nc.vector.tensor_scalar_add(out=den_sb[:], in0=den_ps[:], scalar1=1e-16)
        rec = sb.tile([P, H], F32, name="rec", tag="rec", bufs=2)
        nc.vector.reciprocal(out=rec[:], in_=den_sb[:])
        y_sb = sb.tile([P, D], F32, name="y_sb", tag="y_sb", bufs=2)
        nc.vector.tensor_tensor(
            out=y_sb[:].rearrange("p (h c) -> p h c", h=H),
            in0=out_ps[:].rearrange("p (h c) -> p h c", h=H),
            in1=rec[:].unsqueeze(2).to_broadcast([P, H, D // H]),
            op=ALU.mult,
        )
        if b_rep is not None:
            nc.vector.tensor_add(out=y_sb[:], in0=y_sb[:], in1=b_rep[:])
        # mean and variance sums on the ACT engine (it has spare capacity)
        sq16 = sb.tile([P, D], F16, name="sq16", tag="sq16", bufs=2)
        sy = sb.tile([P, 1], F32, name="sy", tag="sy", bufs=2)
        nc.scalar.activation(sq16[:], y_sb[:], ACT.Copy, accum_out=sy[:, 0:1])
        nmu = sb.tile([P, 1], F32, name="nmu", tag="nmu", bufs=2)
        nc.scalar.mul(nmu[:], sy[:], -1.0 / D)  # nmu = -mu
        ssq = sb.tile([P, 1], F32, name="ssq", tag="ssq", bufs=2)
        nc.scalar.activation(
            sq16[:], y_sb[:], ACT.Square, bias=nmu[:, 0:1],
            accum_out=ssq[:, 0:1],
        )
        # rstd = 1/sqrt(ssq/D + eps) on DVE via bit-trick + 2 Newton steps
        # (avoids Sqrt/Ln on the ACT engine, whose tables clash with Exp)
        vv = sb.tile([P, 1], F32, name="vv", tag="vv", bufs=2)
        nc.vector.tensor_scalar(
            out=vv[:], in0=ssq[:], scalar1=1.0 / D, scalar2=float(EPS),
            op0=ALU.mult, op1=ALU.add,
        )
        ri = sb.tile([P, 1], mybir.dt.int32, name="ri", tag="ri", bufs=2)
        nc.vector.tensor_scalar(
            out=ri[:], in0=vv[:].bitcast(mybir.dt.int32), scalar1=1,
            scalar2=-1, op0=ALU.logical_shift_right, op1=ALU.bitwise_xor,
        )
        nc.vector.tensor_scalar_add(out=ri[:], in0=ri[:], scalar1=0x5F3759DF + 1)
        rstd = sb.tile([P, 1], F32, name="rstd", tag="rstd", bufs=2)
        nc.vector.tensor_copy(out=rstd[:], in_=ri[:].bitcast(F32))
        for _ in range(2):  # 2 Newton steps: rel err ~5e-6
            nr_a = sb.tile([P, 1], F32, name="nr_a", tag="nr_a", bufs=2)
            nc.vector.tensor_mul(out=nr_a[:], in0=rstd[:], in1=rstd[:])
            nc.vector.tensor_mul(out=nr_a[:], in0=nr_a[:], in1=vv[:])
            nc.vector.tensor_scalar(
                out=nr_a[:], in0=nr_a[:], scalar1=-0.5, scalar2=1.5,
                op0=ALU.mult, op1=ALU.add,
            )
            nc.vector.tensor_mul(out=rstd[:], in0=rstd[:], in1=nr_a[:])
        mm = sb.tile([P, 1], F32, name="mm", tag="mm", bufs=2)
        nc.vector.tensor_mul(out=mm[:], in0=nmu[:], in1=rstd[:])
        if g_rep is None and be_rep is None:
            nc.scalar.activation(
                xn_sb[b][:], y_sb[:], ACT.Relu,
                scale=rstd[:, 0:1], bias=mm[:, 0:1],
            )
        else:
            ln_sb = sb.tile([P, D], F32, name="ln_sb", tag="ln_sb", bufs=2)
            nc.scalar.activation(
                ln_sb[:], y_sb[:], ACT.Identity,
                scale=rstd[:, 0:1], bias=mm[:, 0:1],
            )
            if g_rep is not None:
                nc.vector.tensor_mul(out=ln_sb[:], in0=ln_sb[:], in1=g_rep[:])
            if be_rep is not None:
                nc.vector.tensor_add(out=ln_sb[:], in0=ln_sb[:], in1=be_rep[:])
            nc.scalar.activation(xn_sb[b][:], ln_sb[:], ACT.Relu)

    def trans_block(b):
        # xn block -> feature-major xT for the next layer's GEMM
        for k in range(KD):
            t_ps = psum.tile([P, P], F16, name="t_ps", tag="tr", bufs=2)
            nc.tensor.transpose(
                out=t_ps[:], in_=xn_sb[b][:, k * P : (k + 1) * P],
                identity=id128[:],
            )
            nc.vector.tensor_copy(
                out=xT_sb[k][:, b * P : (b + 1) * P], in_=t_ps[:]
            )

    # ---------------- program
    load_weights(0)
    compute_wa()
    for b in range(NB):
        gemm_block(0, b)
        if (b + 1) % BPP == 0:
            ag_piece(0, (b + 1) // BPP - 1)

    pm_pool = ctx.enter_context(tc.tile_pool(name="pm", bufs=2))
    pool_ps = psum.tile([G, D], F32, name="pool_ps", tag="pool", bufs=1)

    def pool_block(b):
        pm_sb = pm_pool.tile([P, G], F16, name="pm_sb", tag="pm_sb", bufs=2)
        nc.scalar.dma_start(out=pm_sb[:], in_=I["poolmask"][b])
        nc.tensor.matmul(
            out=pool_ps[:], lhsT=pm_sb[:], rhs=xn_sb[b][:],
            start=(b == 0), stop=(b == NB - 1),
        )

    for l in range(L):
        if l + 1 < L:
            load_weights(l + 1)
            compute_wa()
        for b in range(NB):
            edge_block(l, b)
            if l + 1 < L:
                trans_block(b)
                gemm_block(l + 1, b)
                if (b + 1) % BPP == 0:
                    ag_piece(l + 1, (b + 1) // BPP - 1)
            else:
                pool_block(b)

    # ---------------- pooling epilogue (counts precomputed on host) + FC
    pool_sb = res.tile([G, D], F32, name="pool_sb")
    nc.vector.tensor_copy(out=pool_sb[:], in_=pool_ps[:])
    nc.sync.dma_start(out=ar_in[:], in_=pool_sb[:])
    nc.gpsimd.collective_compute(
        "AllReduce",
        ALU.add,
        replica_groups=rg,
        ins=[ar_in[:].opt()],
        outs=[ar_out[:].opt()],
    )
    pf_sb = res.tile([G, D], F32, name="pf_sb")
    nc.sync.dma_start(out=pf_sb[:], in_=ar_out[:])
    crec_sb = res.tile([G, 1], F32, name="crec_sb")
    nc.sync.dma_start(out=crec_sb[:], in_=I["crec"][:])
    pn16 = res.tile([G, D], F16, name="pn16")
    nc.vector.tensor_tensor(
        out=pn16[:], in0=pf_sb[:],
        in1=crec_sb[:].to_broadcast([G, D]), op=ALU.mult,
    )
    # transpose pooled -> (KD chunks of (128, G))
    pT_sb = res.tile([P, KD, G], F16, name="pT_sb")
    for k in range(KD):
        t2_ps = psum.tile([P, G], F16, name="t2_ps", tag="tr", bufs=2)
        nc.tensor.transpose(
            out=t2_ps[:], in_=pn16[:, k * P : (k + 1) * P], identity=id128[:]
        )
        nc.vector.tensor_copy(out=pT_sb[:, k, :], in_=t2_ps[:])
    fcw_sb = res.tile([P, KD, OUT], F16, name="fcw_sb")
    nc.sync.dma_start(out=fcw_sb[:], in_=I["fc_W"][:])
    fc_ps = psum.tile([G, OUT], F32, name="fc_ps", tag="big", bufs=2)
    for k in range(KD):
        nc.tensor.matmul(
            out=fc_ps[:], lhsT=pT_sb[:, k, :], rhs=fcw_sb[:, k, :],
            start=(k == 0), stop=(k == KD - 1),
        )
    o_sb = res.tile([G, OUT], F32, name="o_sb")
    if not meta["skip_fcb"]:
        fcb_rep = res.tile([P, OUT], F32, name="fcb_rep")
        nc.sync.dma_start(out=fcb_rep[:], in_=I["fcb_rep"][:])
        nc.vector.tensor_add(out=o_sb[:], in0=fc_ps[:], in1=fcb_rep[0:G, :])
    else:
        nc.vector.tensor_copy(out=o_sb[:], in_=fc_ps[:])
    nc.sync.dma_start(out=out_ap[:], in_=o_sb[:])
    ctx.close()


# --------------------------------------------------------------------------
# Entry point
# --------------------------------------------------------------------------


def kernel(**inputs):
    global LAST_RESULTS
    cfg = _full_cfg()
    in_maps, meta = _prep(inputs, cfg)

    nc = bacc.Bacc(
        "TRN2",
        target_bir_lowering=False,
        debug=False,
        enable_asserts=False,
        num_devices=cfg["n_cores"],
        num_swdge_queues=cfg["NQ"],
    )
    I = {}
    for name, arr in in_maps[0].items():
        I[name] = nc.dram_tensor(
            name, arr.shape, mybir.dt.from_np(arr.dtype), kind="ExternalInput"
        ).ap()
    out_ap = nc.dram_tensor(
        "out", (cfg["G"], cfg["OUT"]), F32, kind="ExternalOutput"
    ).ap()

    with tile.TileContext(nc) as tc:
        build(tc, cfg, meta, I, out_ap)
    nc.compile()

    trace = bool(int(os.environ.get("GAT_TRACE", "0")))
    res = run_bass_kernel_spmd(
        nc,
        in_maps,
        core_ids=list(range(cfg["n_cores"])),
        trace=trace,
    )
    LAST_RESULTS = res
    return np.asarray(res.results[0]["out"])
